# revision 1
# baseline (speedup 1.0000x reference)
"""Trainium2 Bass kernel for a dense transformer block (attention + MLP, 2 LayerNorms).

Sharding: pure data-parallel over 8 cores, one shard per (batch, half-sequence):
core 2*b + h handles queries for tokens [h*512, (h+1)*512) of batch b. Each core
recomputes K/V for its full causal context (prefix + own tokens) so no
collectives are needed; the causal mask is shipped as per-core data.

On-chip layout is feature-major (features on partitions, tokens on the free
axis): all biases / LN affine params are per-partition scalars that fuse into
eviction instructions. All big matmuls run as float32r (full PE rate at N=512
with fp32 operands). Weights and x are pre-tiled on the host so every weight
DMA is a single contiguous transfer.
"""

from contextlib import ExitStack

import numpy as np

import concourse.bacc as bacc
import concourse.bass as bass
import concourse.tile as tile
from concourse import mybir
from concourse.bass_utils import run_bass_kernel_spmd
from concourse.masks import make_identity

B, S, D, H = 4, 1024, 1024, 16
DH = D // H
EPS = 1e-5
TOK = 512   # queries per core
CTX = 1024  # context tokens per core
P = 128
F32 = mybir.dt.float32
F32R = mybir.dt.float32r
AF = mybir.ActivationFunctionType
OP = mybir.AluOpType

N_CORES = 8


def _r(ap):
    """View an fp32 AP as float32r for full-rate PE matmuls."""
    return ap.bitcast(F32R)


def _mm(nc, out, lhsT, rhs, start, stop, tile_position=None):
    nc.tensor.matmul(out, _r(lhsT), _r(rhs), start=start, stop=stop,
                     tile_position=tile_position)


def build_block_kernel(nc, tc, io):
    ctx = ExitStack()
    (xt, wq4, wk4, wv3, b_qkv, wat4, b_attn, ln1_g, ln1_b, wfc4, b_fc,
     wmlp4, b_mlp, ln2_g, ln2_b, maskT, out) = io

    const = ctx.enter_context(tc.tile_pool(name="const", bufs=1))

    ident = const.tile([P, P], F32)
    make_identity(nc, ident)
    ident_r = const.tile([P, P], F32R)
    nc.scalar.copy(out=ident_r, in_=ident)
    ones_f = const.tile([P, P], F32)
    nc.vector.memset(ones_f, 1.0)
    ones_t = const.tile([P, P], F32R)
    nc.scalar.copy(out=ones_t, in_=ones_f)

    ps_big = ctx.enter_context(tc.tile_pool(name="ps_big", bufs=3, space="PSUM"))

    xa_stack = ExitStack()
    xa_pool = xa_stack.enter_context(tc.tile_pool(name="xa_pool", bufs=1))
    X_f = xa_pool.tile([P, 8, CTX], F32R)        # x^T, feature-major
    a_all = xa_pool.tile([P, 8, TOK], F32R)      # attention output^T (normalized)

    v_stack = ExitStack()
    v_pool = v_stack.enter_context(tc.tile_pool(name="v_pool", bufs=1))
    V_sb = v_pool.tile([P, 8, H, DH + 1], F32R)   # [V | 1] per head, token-major
    nc.scalar.copy(
        out=V_sb[:, :, :, DH:DH + 1],
        in_=ones_f.rearrange("p (a b c) -> p a b c", a=8, b=H))

    # ============ phase 0: load x^T (host-pretiled), compute V ============
    with tc.tile_pool(name="wv_pool", bufs=1) as wv_pool:
        # wv first half, then X' column blocks (two queues), then wv 2nd half
        wv_t = wv_pool.tile([P, 8, D], F32R)
        nc.sync.dma_start(out=wv_t[:, :, 0:TOK], in_=wv3[:, :, 0:TOK])
        for tt in range(8):
            eng = nc.sync if tt % 2 == 0 else nc.gpsimd
            eng.dma_start(out=X_f[:, :, tt * P:(tt + 1) * P], in_=xt[tt])
        nc.sync.dma_start(out=wv_t[:, :, TOK:], in_=wv3[:, :, TOK:])

        def col_param(src_ap, n_tiles, name):
            t = const.tile([P, n_tiles], F32, name=name)
            nc.sync.dma_start(out=t, in_=src_ap.rearrange("(t p) -> p t", p=P))
            return t

        bq_s = col_param(b_qkv[0:D], 8, "bq_s")
        bq_sc = const.tile([P, 8], F32)
        nc.vector.tensor_scalar_mul(out=bq_sc, in0=bq_s,
                                    scalar1=float(1.0 / np.sqrt(DH)))
        bk_s = col_param(b_qkv[D:2 * D], 8, "bk_s")
        battn_s = col_param(b_attn, 8, "battn_s")
        ln1g_s = col_param(ln1_g, 8, "ln1g_s")
        ln1b_s = col_param(ln1_b, 8, "ln1b_s")
        bfc_s = col_param(b_fc, 32, "bfc_s")
        bmlp_s = col_param(b_mlp, 8, "bmlp_s")
        ln2g_s = col_param(ln2_g, 8, "ln2g_s")
        ln2b_s = col_param(ln2_b, 8, "ln2b_s")
        eps_c = const.tile([P, 1], F32)
        nc.vector.memset(eps_c, EPS)
        bv_b = const.tile([P, D], F32)
        bv_src = b_qkv[2 * D:3 * D]
        nc.sync.dma_start(
            out=bv_b,
            in_=bass.AP(tensor=bv_src.tensor, offset=bv_src.offset,
                        ap=[[0, P]] + list(bv_src.ap)))

        for tt in range(8):
            # V rows for token-tile tt ready once X_f[:, :, tt-cols] arrives
            for half in range(2):
                psV = ps_big.tile([P, TOK], F32, tag="ps")
                for dk in range(8):
                    _mm(nc, psV, X_f[:, dk, tt * P:(tt + 1) * P],
                        wv_t[:, dk, half * TOK:(half + 1) * TOK],
                        start=(dk == 0), stop=(dk == 7))
                nc.vector.scalar_tensor_tensor(
                    out=V_sb[:, tt, half * 8:(half + 1) * 8, 0:DH],
                    in0=psV.rearrange("p (h d) -> p h d", d=DH),
                    scalar=0.0, in1=bv_b[:, half * TOK:(half + 1) * TOK]
                    .rearrange("p (h d) -> p h d", d=DH),
                    op0=OP.add, op1=OP.add)

    # ============== attention, one head-pair at a time ==============
    with tc.tile_pool(name="wqk", bufs=2) as wqk_pool, \
            tc.tile_pool(name="q_pool", bufs=3) as q_pool, \
            tc.tile_pool(name="k_pool", bufs=3) as k_pool, \
            tc.tile_pool(name="p_pool", bufs=3) as p_pool, \
            tc.tile_pool(name="m_pool", bufs=1) as m_pool, \
            tc.tile_pool(name="att_sm", bufs=2) as att_sm, \
            tc.tile_pool(name="ps_acc", bufs=2, space="PSUM") as ps_acc, \
            tc.tile_pool(name="ps_d", bufs=2, space="PSUM") as ps_d:

        mask01 = m_pool.tile([P, 8, TOK], F32)
        nc.sync.dma_start(out=mask01, in_=maskT)

        for hp in range(8):
            wq_t = wqk_pool.tile([P, 8, P], F32R, tag="wq")
            nc.sync.dma_start(out=wq_t, in_=wq4[hp])
            wk_t = wqk_pool.tile([P, 8, P], F32R, tag="wk")
            nc.sync.dma_start(out=wk_t, in_=wk4[hp])

            psQ = ps_big.tile([P, TOK], F32, tag="ps")
            for dk in range(8):
                _mm(nc, psQ, wq_t[:, dk, :], X_f[:, dk, TOK:CTX],
                    start=(dk == 0), stop=(dk == 7))
            q_t = q_pool.tile([P, TOK], F32R, tag="q")
            # fold the 1/sqrt(dh) softmax scale into Q (DVE; ACT stays on Exp)
            nc.vector.tensor_scalar(
                out=q_t, in0=psQ, scalar1=float(1.0 / np.sqrt(DH)),
                scalar2=bq_sc[:, hp:hp + 1], op0=OP.mult, op1=OP.add)

            k_t = k_pool.tile([P, CTX], F32R, tag="k")
            for half in range(2):
                psK = ps_big.tile([P, TOK], F32, tag="ps")
                for dk in range(8):
                    _mm(nc, psK, wk_t[:, dk, :],
                        X_f[:, dk, half * TOK:(half + 1) * TOK],
                        start=(dk == 0), stop=(dk == 7))
                nc.vector.tensor_scalar_add(
                    out=k_t[:, half * TOK:(half + 1) * TOK], in0=psK,
                    scalar1=bk_s[:, hp:hp + 1])

            pA = p_pool.tile([P, 8, TOK], F32R, tag="p")
            pB = p_pool.tile([P, 8, TOK], F32R, tag="p")
            # context tile kt >= 4 holds own tokens (kt-4)*128.. which only
            # queries q >= (kt-4)*128 can see -> restrict columns
            qstart = [0, 0, 0, 0, 0, 128, 256, 384]
            for kt in range(8):
                qs = qstart[kt]
                psSA = ps_big.tile([P, TOK], F32, tag="ps")
                psSB = ps_big.tile([P, TOK], F32, tag="ps")
                _mm(nc, psSA[:, qs:], k_t[0:64, kt * P:(kt + 1) * P],
                    q_t[0:64, qs:], start=True, stop=True, tile_position=(0, 0))
                _mm(nc, psSB[:, qs:], k_t[64:128, kt * P:(kt + 1) * P],
                    q_t[64:128, qs:], start=True, stop=True,
                    tile_position=(64, 0))
                nc.scalar.activation(pA[:, kt, qs:], psSA[:, qs:], AF.Exp)
                nc.scalar.activation(pB[:, kt, qs:], psSB[:, qs:], AF.Exp)
                nc.vector.tensor_mul(pA[:, kt, qs:], pA[:, kt, qs:],
                                     mask01[:, kt, qs:])
                nc.vector.tensor_mul(pB[:, kt, qs:], pB[:, kt, qs:],
                                     mask01[:, kt, qs:])

            psA = ps_acc.tile([65, TOK], F32, tag="acc")
            psB = ps_acc.tile([65, TOK], F32, tag="acc")
            for kt in range(8):
                qs = qstart[kt]
                _mm(nc, psA[:, qs:], V_sb[:, kt, 2 * hp, :], pA[:, kt, qs:],
                    start=(kt == 0), stop=(kt == 7))
                _mm(nc, psB[:, qs:], V_sb[:, kt, 2 * hp + 1, :], pB[:, kt, qs:],
                    start=(kt == 0), stop=(kt == 7))

            # normalize: row 64 of psA/psB is the softmax denominator
            dtmp = att_sm.tile([65, 2, TOK], F32R, tag="dtmp", bufs=1)
            with nc.allow_low_precision(reason="float32r is 4-byte"):
                nc.vector.reciprocal(out=dtmp[64:65, 0, :], in_=psA[64:65, :])
                nc.vector.reciprocal(out=dtmp[64:65, 1, :], in_=psB[64:65, :])
            # hop both reciprocal rows to partition 0 in one small DMA, then
            # broadcast across 64 partitions on POOL
            drow = att_sm.tile([1, 2, TOK], F32R, tag="drow", bufs=1)
            nc.gpsimd.dma_start(out=drow[0:1, :, :], in_=dtmp[64:65, :, :])
            rb = att_sm.tile([64, 2, TOK], F32R, tag="rb")
            nc.gpsimd.partition_broadcast(rb[:, 0, :], drow[0:1, 0, :],
                                          channels=64)
            nc.gpsimd.partition_broadcast(rb[:, 1, :], drow[0:1, 1, :],
                                          channels=64)
            nc.vector.scalar_tensor_tensor(
                out=a_all[0:64, hp, :], in0=psA[0:64, :], scalar=0.0,
                in1=rb[:, 0, :], op0=OP.add, op1=OP.mult)
            btmp = att_sm.tile([64, TOK], F32R, tag="btmp")
            nc.vector.scalar_tensor_tensor(
                out=btmp, in0=psB[0:64, :], scalar=0.0,
                in1=rb[:, 1, :], op0=OP.add, op1=OP.mult)
            nc.gpsimd.dma_start(out=a_all[64:128, hp, :], in_=btmp)

    v_stack.close()  # V dead after the last a@v

    # r1 allocated only now (keeps attention-phase SBUF free); right-side
    # stack so its lifetime may straddle the left-stack pool closes
    r1_pool = ctx.enter_context(tc.tile_pool(name="r1_pool", bufs=1, side="right"))
    r1 = r1_pool.tile([P, 8, TOK], F32R)

    # ================= attn projection + residual =================
    with tc.tile_pool(name="wattn", bufs=2) as wattn_pool:
        for mt in range(8):
            wat = wattn_pool.tile([P, 8, P], F32R, tag="wat")
            nc.sync.dma_start(out=wat, in_=wat4[mt])
            psO = ps_big.tile([P, TOK], F32, tag="ps")
            for j in range(8):
                _mm(nc, psO, wat[:, j, :], a_all[:, j, :],
                    start=(j == 0), stop=(j == 7))
            nc.vector.scalar_tensor_tensor(
                out=r1[:, mt, :], in0=psO, scalar=battn_s[:, mt:mt + 1],
                in1=X_f[:, mt, TOK:CTX], op0=OP.add, op1=OP.add)

    xa_stack.close()  # X', a_all dead

    def layer_norm(src, dst, g_s, b_s):
        """dst = g * (src - mean) / sqrt(std + eps) + b, stats over the 1024
        features (partition direction, 8 tiles). Scalar math happens on
        single-partition rows; one matmul broadcasts (mean | rstd) to all
        partitions."""
        with tc.tile_pool(name="ln_sb", bufs=2) as ln_sb, \
                tc.tile_pool(name="ln_one", bufs=1) as ln_one, \
                tc.tile_pool(name="ps_stat", bufs=2, space="PSUM") as ps_stat:
            psSum = ps_stat.tile([1, TOK], F32, tag="st")
            psSq = ps_stat.tile([1, TOK], F32, tag="st")
            for mt in range(8):
                _mm(nc, psSum, ones_t[:, 0:1], src[:, mt, :],
                    start=(mt == 0), stop=(mt == 7))
                sq_t = ln_sb.tile([P, TOK], F32R, tag="sq")
                nc.vector.tensor_mul(sq_t, src[:, mt, :], src[:, mt, :])
                _mm(nc, psSq, ones_t[:, 0:1], sq_t,
                    start=(mt == 0), stop=(mt == 7))
            # row-wise scalar math on partition 0: mean, unbiased var,
            # rstd' = 1/sqrt(std + eps)
            mr = ln_one.tile([1, 2, TOK], F32R)   # (mean | rstd') row
            t_r = ln_one.tile([1, 2, TOK], F32)
            nc.vector.tensor_scalar_mul(out=mr[0:1, 0, :], in0=psSum,
                                        scalar1=float(1.0 / D))
            nc.vector.tensor_scalar_mul(out=t_r[0:1, 1, :], in0=psSq,
                                        scalar1=float(1.0 / D))
            nc.vector.tensor_mul(t_r[0:1, 0, :], mr[0:1, 0, :], mr[0:1, 0, :])
            nc.vector.tensor_sub(t_r[0:1, 0, :], t_r[0:1, 1, :], t_r[0:1, 0, :])
            nc.scalar.activation(t_r[0:1, 1, :], t_r[0:1, 0, :], AF.Sqrt,
                                 scale=float(D / (D - 1.0)))
            nc.scalar.activation(t_r[0:1, 0, :], t_r[0:1, 1, :], AF.Sqrt,
                                 bias=eps_c[0:1])
            with nc.allow_low_precision(reason="float32r is 4-byte"):
                nc.vector.reciprocal(mr[0:1, 1, :], t_r[0:1, 0, :])
            # broadcast rows to all partitions: psMR[:, 0, :] = mean,
            # psMR[:, 1, :] = rstd'  (N capped at 512 for 4-byte matmuls)
            psMR = ps_stat.tile([P, 2, TOK], F32, tag="psmr", bufs=1)
            _mm(nc, psMR[:, 0, :], ones_t[0:1, :], mr[0:1, 0, :],
                start=True, stop=True)
            _mm(nc, psMR[:, 1, :], ones_t[0:1, :], mr[0:1, 1, :],
                start=True, stop=True)
            mean_b = ln_one.tile([P, TOK], F32)
            nc.vector.tensor_copy(out=mean_b, in_=psMR[:, 0, :])
            rs_b = ln_one.tile([P, TOK], F32)
            nc.vector.tensor_copy(out=rs_b, in_=psMR[:, 1, :])
            for mt in range(8):
                e = nc.vector if mt % 2 == 0 else nc.gpsimd
                t1 = ln_sb.tile([P, TOK], F32, tag="t1")
                e.tensor_sub(t1, src[:, mt, :], mean_b)
                nc.vector.scalar_tensor_tensor(
                    out=dst[:, mt, :], in0=t1, scalar=g_s[:, mt:mt + 1],
                    in1=rs_b, op0=OP.mult, op1=OP.mult)
                nc.vector.tensor_scalar_add(
                    out=dst[:, mt, :], in0=dst[:, mt, :],
                    scalar1=b_s[:, mt:mt + 1])

    with tc.tile_pool(name="h1_pool", bufs=1) as h1_pool:
        h1 = h1_pool.tile([P, 8, TOK], F32R)
        layer_norm(r1, h1, ln1g_s, ln1b_s)

        # ================= MLP =================
        with tc.tile_pool(name="r2y", bufs=1) as r2y_pool:
            r2 = r2y_pool.tile([P, 8, TOK], F32R)
            with tc.tile_pool(name="m1_pool", bufs=1) as m1_pool, \
                    tc.tile_pool(name="wfc", bufs=4) as wfc_pool, \
                    tc.tile_pool(name="wmlp", bufs=3) as wmlp_pool:
                m1 = m1_pool.tile([P, 32, TOK], F32R)
                for mt in range(32):
                    wfc_t = wfc_pool.tile([P, 8, P], F32R, tag="wfc")
                    nc.sync.dma_start(out=wfc_t, in_=wfc4[mt])
                    psF = ps_big.tile([P, TOK], F32, tag="ps")
                    for dk in range(8):
                        _mm(nc, psF, wfc_t[:, dk, :], h1[:, dk, :],
                            start=(dk == 0), stop=(dk == 7))
                    nc.scalar.activation(m1[:, mt, :], psF, AF.Relu,
                                         bias=bfc_s[:, mt:mt + 1], scale=1.0)
                for mt in range(8):
                    wmlp_t = wmlp_pool.tile([P, 32, P], F32R, tag="wmlp")
                    nc.sync.dma_start(out=wmlp_t, in_=wmlp4[mt])
                    psM = ps_big.tile([P, TOK], F32, tag="ps")
                    for k4 in range(32):
                        _mm(nc, psM, wmlp_t[:, k4, :], m1[:, k4, :],
                            start=(k4 == 0), stop=(k4 == 31))
                    nc.vector.scalar_tensor_tensor(
                        out=r2[:, mt, :], in0=psM, scalar=bmlp_s[:, mt:mt + 1],
                        in1=h1[:, mt, :], op0=OP.add, op1=OP.add)

            y = r2y_pool.tile([P, 8, TOK], F32R)
            layer_norm(r2, y, ln2g_s, ln2b_s)

            # ================= transpose back + store =================
            with tc.tile_pool(name="out_tm", bufs=2) as out_pool:
                for tt in range(4):
                    o_tm = out_pool.tile([P, D], F32, tag="otm")
                    for dt in range(8):
                        psT = ps_big.tile([P, P], F32R, tag="ps")
                        nc.tensor.transpose(
                            psT, y[:, dt, tt * P:(tt + 1) * P], ident_r)
                        nc.vector.tensor_copy(out=o_tm[:, dt * P:(dt + 1) * P],
                                              in_=psT)
                    nc.sync.dma_start(out=out[tt * P:(tt + 1) * P, :], in_=o_tm)

    ctx.close()


_BUILT = None


def _build():
    global _BUILT
    if _BUILT is not None:
        return _BUILT
    nc = bacc.Bacc("TRN2", target_bir_lowering=False, debug=False,
                   enable_asserts=False, num_devices=N_CORES)

    def din(name, shape, dtype=F32):
        return nc.dram_tensor(name, list(shape), dtype, kind="ExternalInput").ap()

    xt = din("xt", (8, P, 8, P), F32R)          # [tt, p, dt, m]
    wq4 = din("wq4", (8, P, 8, P), F32R)        # [hp, p, dk, m]
    wk4 = din("wk4", (8, P, 8, P), F32R)
    wv3 = din("wv3", (P, 8, D), F32R)           # [p, dk, m]
    b_qkv = din("b_qkv", (3 * D,))
    wat4 = din("wat4", (8, P, 8, P), F32R)      # [mt, p, j, m]
    b_attn = din("b_attn_proj", (D,))
    ln1_g = din("ln1_g", (D,))
    ln1_b = din("ln1_b", (D,))
    wfc4 = din("wfc4", (32, P, 8, P), F32R)     # [mt, p, dk, m]
    b_fc = din("b_fc", (4 * D,))
    wmlp4 = din("wmlp4", (8, P, 32, P), F32R)   # [mt, p, k4, m]
    b_mlp = din("b_mlp_proj", (D,))
    ln2_g = din("ln2_g", (D,))
    ln2_b = din("ln2_b", (D,))
    maskT = din("maskT", (P, 8, TOK))           # [p, kt, q]
    out_h = nc.dram_tensor("out", [TOK, D], F32, kind="ExternalOutput")

    io = [xt, wq4, wk4, wv3, b_qkv, wat4, b_attn, ln1_g, ln1_b, wfc4, b_fc,
          wmlp4, b_mlp, ln2_g, ln2_b, maskT, out_h.ap()]
    with tile.TileContext(nc) as tc:
        build_block_kernel(nc, tc, io)
    nc.compile()
    _BUILT = nc
    return nc


def _tile4(w, n_in, n_out):
    """[K, M] weight -> [n_out tiles, P, n_in tiles, P]: t4[mt, p, k, m] =
    w[k*P + p, mt*P + m]."""
    K, M = w.shape
    assert K == n_in * P and M == n_out * P
    return np.ascontiguousarray(
        w.reshape(n_in, P, n_out, P).transpose(2, 1, 0, 3))


def _in_maps(inputs):
    f32 = lambda a: np.ascontiguousarray(np.asarray(a), dtype=np.float32)
    x = f32(inputs["x"])
    w_qkv = f32(inputs["w_qkv"])
    shared = {
        "wq4": _tile4(w_qkv[:, 0:D], 8, 8),
        "wk4": _tile4(w_qkv[:, D:2 * D], 8, 8),
        "wv3": np.ascontiguousarray(
            w_qkv[:, 2 * D:].reshape(8, P, D).transpose(1, 0, 2)),
        "wat4": _tile4(f32(inputs["w_attn_proj"]), 8, 8),
        "wfc4": _tile4(f32(inputs["w_fc"]), 8, 32),
        "wmlp4": _tile4(f32(inputs["w_mlp_proj"]), 32, 8),
        "b_qkv": f32(inputs["b_qkv"]),
        "b_attn_proj": f32(inputs["b_attn_proj"]),
        "ln1_g": f32(inputs["ln1_g"]), "ln1_b": f32(inputs["ln1_b"]),
        "b_fc": f32(inputs["b_fc"]),
        "b_mlp_proj": f32(inputs["b_mlp_proj"]),
        "ln2_g": f32(inputs["ln2_g"]), "ln2_b": f32(inputs["ln2_b"]),
    }
    tri = (np.arange(TOK)[:, None] <= np.arange(TOK)[None, :]).astype(np.float32)
    maps = []
    for b in range(B):
        for hh in range(2):
            x_core = np.zeros((CTX, D), np.float32)
            if hh == 1:
                x_core[:TOK] = x[b, :TOK]
            x_core[TOK:] = x[b, hh * TOK:(hh + 1) * TOK]
            xT = x_core.T                      # [D, CTX]
            xt = np.ascontiguousarray(         # [tt, p, dt, m]
                xT.reshape(8, P, 8, P).transpose(2, 1, 0, 3))
            maskT = np.zeros((CTX, TOK), np.float32)
            maskT[:TOK] = float(hh)
            maskT[TOK:] = tri
            mask3 = np.ascontiguousarray(      # [p, kt, q]
                maskT.reshape(8, P, TOK).transpose(1, 0, 2))
            maps.append({"xt": xt, "maskT": mask3, **shared})
    return maps


def run_on_cores(inputs, trace=False, **kwargs):
    """Run the SPMD kernel; returns (full_output, BassKernelResults)."""
    nc = _build()
    maps = _in_maps(inputs)
    res = run_bass_kernel_spmd(nc, maps, core_ids=list(range(N_CORES)),
                               trace=trace, **kwargs)
    out = np.zeros((B, S, D), np.float32)
    for c in range(N_CORES):
        b, hh = divmod(c, 2)
        out[b, hh * TOK:(hh + 1) * TOK] = res.results[c]["out"]
    return out, res


def kernel(**inputs) -> np.ndarray:
    out, _ = run_on_cores(inputs, trace=False)
    return out



# revision 19
# speedup vs baseline: 1.4291x; 1.4291x over previous
"""Trainium2 Bass kernel for a dense transformer block (attention + MLP, 2 LayerNorms).

Sharding: pure data-parallel over 8 cores, one shard per (batch, half-sequence):
core 2*b + h handles queries for tokens [h*512, (h+1)*512) of batch b. Each core
recomputes K/V for its full causal context; no collectives. The junk prefix on
even cores (h=0) is neutralized with per-core data (zeroed V bias + zeroed
denominator column), so no full-tile mask multiplies are needed — only the 8
diagonal [128,128] triangular blocks get a mask multiply.

Everything on the matmul path is bf16 (PE is N-column-bound, so bf16 matches
fp32r peak on big tiles and is 4x faster on the N=128 diagonal tiles, while
halving DMA bytes and doubling DVE throughput). PSUM accumulation is fp32.
LayerNorm row stats are DMA-scattered to [128, x] tiles so all scalar math
runs full-lane; mean/rstd broadcast back via a 1-row PE matmul.
"""

from contextlib import ExitStack

import ml_dtypes
import numpy as np

import concourse.bacc as bacc
import concourse.bass as bass
import concourse.tile as tile
from concourse import mybir
from concourse.bass_utils import run_bass_kernel_spmd

B, S, D, H = 4, 1024, 1024, 16
DH = D // H
EPS = 1e-5
TOK = 512   # queries per core
CTX = 1024  # context tokens per core
P = 128
F32 = mybir.dt.float32
F32R = mybir.dt.float32r
BF16 = mybir.dt.bfloat16
AF = mybir.ActivationFunctionType
OP = mybir.AluOpType

N_CORES = 8
DEBUG = False
QSTART = [0, 0, 0, 0, 0, 128, 256, 384]   # first query col computed per ctx tile
TT_ORDER = [4, 5, 6, 7, 0, 1, 2, 3]       # own-token tiles first (early PE work)


def _r(ap):
    """View an fp32 AP as float32r for full-rate PE matmuls."""
    return ap.bitcast(F32R)


def build_block_kernel(nc, tc, io):
    ctx = ExitStack()
    (xt, wq_all_h, wk_all_h, wv3, b_qkv, wat_all_h, b_attn, ln1_g, ln1_b,
     wfc4, b_fc, wmlp4, b_mlp, ln2_g, ln2_b, tri_h, vcol_h, bvA_h, bvB_h,
     sel16_h, out, dbg_a, dbg_dd, dbg_st, dbg_r1, dbg_h1) = io

    const = ctx.enter_context(tc.tile_pool(name="const", bufs=1))

    ones_row = const.tile([1, P], F32)       # lhsT for row->all-partition bcast
    nc.vector.memset(ones_row, 1.0)
    ones_bf = const.tile([P, 1], BF16)       # lhsT for column-sum stats
    nc.vector.memset(ones_bf, 1.0)
    sel16 = const.tile([H, 8, P], BF16)      # per-head-pair selector lhsT
    nc.sync.dma_start(out=sel16, in_=sel16_h)
    eps_c = const.tile([P, 1], F32)
    nc.vector.memset(eps_c, EPS)

    tri = const.tile([P, P], BF16)           # tri[m, n] = n >= m
    nc.sync.dma_start(out=tri, in_=tri_h)
    vcol = const.tile([P, 8 * H], BF16)      # per-core denominator column
    nc.sync.dma_start(out=vcol, in_=vcol_h)
    bvA = const.tile([P, D], BF16)           # v-bias for ctx tiles 0..3
    bvB = const.tile([P, D], BF16)           # v-bias for ctx tiles 4..7
    for bv_t, bv_src in ((bvA, bvA_h), (bvB, bvB_h)):
        nc.sync.dma_start(
            out=bv_t,
            in_=bass.AP(tensor=bv_src.tensor, offset=bv_src.offset,
                        ap=[[0, P]] + list(bv_src.ap)))

    def col_param(src_ap, n_tiles, name):
        t = const.tile([P, n_tiles], F32, name=name)
        nc.sync.dma_start(out=t, in_=src_ap.rearrange("(t p) -> p t", p=P))
        return t

    bq_s = col_param(b_qkv[0:D], 8, "bq_s")
    bq_sc = const.tile([P, 8], F32)
    nc.vector.tensor_scalar_mul(out=bq_sc, in0=bq_s,
                                scalar1=float(1.0 / np.sqrt(DH)))
    bk_s = col_param(b_qkv[D:2 * D], 8, "bk_s")
    battn_s = col_param(b_attn, 8, "battn_s")
    ln1g_s = col_param(ln1_g, 8, "ln1g_s")
    ln1b_s = col_param(ln1_b, 8, "ln1b_s")
    bfc_s = col_param(b_fc, 32, "bfc_s")
    bmlp_s = col_param(b_mlp, 8, "bmlp_s")
    ln2g_s = col_param(ln2_g, 8, "ln2g_s")
    ln2b_s = col_param(ln2_b, 8, "ln2b_s")

    ps_big = ctx.enter_context(tc.tile_pool(name="ps_big", bufs=4, space="PSUM"))

    xa_pool = ctx.enter_context(tc.tile_pool(name="xa_pool", bufs=1))
    X_f = xa_pool.tile([P, 8, CTX], BF16)        # x^T, feature-major

    a_pool = ctx.enter_context(tc.tile_pool(name="a_pool", bufs=1))
    a_all = a_pool.tile([P, 8, TOK], BF16)       # attention out^T per head-pair

    att_stack = ExitStack()
    wqk_pool = att_stack.enter_context(tc.tile_pool(name="wqk", bufs=1))
    v_pool = att_stack.enter_context(tc.tile_pool(name="v_pool", bufs=1))
    den_pool = att_stack.enter_context(tc.tile_pool(name="den", bufs=1))
    q_pool = att_stack.enter_context(tc.tile_pool(name="q_pool", bufs=2))
    k_pool = att_stack.enter_context(tc.tile_pool(name="k_pool", bufs=2))
    p_pool = att_stack.enter_context(tc.tile_pool(name="p_pool", bufs=8))
    bt_pool = att_stack.enter_context(tc.tile_pool(name="bt_pool", bufs=2))
    dst_pool = att_stack.enter_context(tc.tile_pool(name="dst_pool", bufs=4))
    ps_acc = att_stack.enter_context(
        tc.tile_pool(name="ps_acc", bufs=4, space="PSUM"))

    V_sb = v_pool.tile([P, 8, H, DH + 1], BF16)  # [V | den-col] token-major
    den16 = den_pool.tile([H, TOK], BF16)        # head h denominator on part h

    # ============ phase 0: load x^T + weights, compute V ============
    with tc.tile_pool(name="wv_pool", bufs=1) as wv_pool:
        wv_t = wv_pool.tile([P, 8, D], BF16)
        nc.sync.dma_start(out=wv_t[:, :, 0:TOK], in_=wv3[:, :, 0:TOK])
        for i, tt in enumerate(TT_ORDER):
            eng = nc.sync if i % 2 == 0 else nc.gpsimd
            eng.dma_start(out=X_f[:, :, tt * P:(tt + 1) * P], in_=xt[tt])
        nc.sync.dma_start(out=wv_t[:, :, TOK:], in_=wv3[:, :, TOK:])
        wq_all = wqk_pool.tile([P, 8, 8, P], BF16)
        nc.gpsimd.dma_start(out=wq_all, in_=wq_all_h)
        wk_all = wqk_pool.tile([P, 8, 8, P], BF16)
        nc.gpsimd.dma_start(out=wk_all, in_=wk_all_h)

        # denominator column (zeroed on prefix tiles for h=0 cores)
        nc.scalar.copy(out=V_sb[:, :, :, DH:DH + 1],
                       in_=vcol.rearrange("p (a b c) -> p a b c", a=8, b=H))

        for tt in TT_ORDER:
            bv_t = bvA if tt < 4 else bvB
            for half in range(2):
                psV = ps_big.tile([P, TOK], F32, tag="ps")
                for dk in range(8):
                    nc.tensor.matmul(psV, X_f[:, dk, tt * P:(tt + 1) * P],
                                     wv_t[:, dk, half * TOK:(half + 1) * TOK],
                                     start=(dk == 0), stop=(dk == 7))
                nc.vector.scalar_tensor_tensor(
                    out=V_sb[:, tt, half * 8:(half + 1) * 8, 0:DH],
                    in0=psV.rearrange("p (h d) -> p h d", d=DH),
                    scalar=0.0, in1=bv_t[:, half * TOK:(half + 1) * TOK]
                    .rearrange("p (h d) -> p h d", d=DH),
                    op0=OP.add, op1=OP.add)

    # wat loads during attention so attn-proj starts immediately after
    wat_pool = ctx.enter_context(tc.tile_pool(name="wat", bufs=1, side="right"))
    wat_all = wat_pool.tile([P, 8, 8, P], BF16)
    nc.sync.dma_start(out=wat_all, in_=wat_all_h)

    # ============== attention, one head-pair at a time ==============
    for hp in range(8):
        psQ = ps_big.tile([P, TOK], F32, tag="ps")
        for dk in range(8):
            nc.tensor.matmul(psQ, wq_all[:, hp, dk, :], X_f[:, dk, TOK:CTX],
                             start=(dk == 0), stop=(dk == 7))
        q_t = q_pool.tile([P, TOK], BF16, tag="q")
        # fold the 1/sqrt(dh) softmax scale into Q
        nc.vector.tensor_scalar(
            out=q_t, in0=psQ, scalar1=float(1.0 / np.sqrt(DH)),
            scalar2=bq_sc[:, hp:hp + 1], op0=OP.mult, op1=OP.add)

        k_t = k_pool.tile([P, CTX], BF16, tag="k")
        for half in range(2):
            psK = ps_big.tile([P, TOK], F32, tag="ps")
            for dk in range(8):
                nc.tensor.matmul(psK, wk_all[:, hp, dk, :],
                                 X_f[:, dk, half * TOK:(half + 1) * TOK],
                                 start=(dk == 0), stop=(dk == 7))
            nc.vector.tensor_scalar_add(
                out=k_t[:, half * TOK:(half + 1) * TOK], in0=psK,
                scalar1=bk_s[:, hp:hp + 1])

        psA = ps_acc.tile([65, TOK], F32, tag="acc")
        psB = ps_acc.tile([65, TOK], F32, tag="acc")
        for kt in range(8):
            qs = QSTART[kt]
            psSA = ps_big.tile([P, TOK], F32, tag="ps")
            psSB = ps_big.tile([P, TOK], F32, tag="ps")
            nc.tensor.matmul(psSA[:, qs:], k_t[0:64, kt * P:(kt + 1) * P],
                             q_t[0:64, qs:], start=True, stop=True,
                             tile_position=(0, 0))
            nc.tensor.matmul(psSB[:, qs:], k_t[64:128, kt * P:(kt + 1) * P],
                             q_t[64:128, qs:], start=True, stop=True,
                             tile_position=(64, 0))
            pa = p_pool.tile([P, TOK], BF16, tag="p")
            pb = p_pool.tile([P, TOK], BF16, tag="p")
            nc.scalar.activation(pa[:, qs:], psSA[:, qs:], AF.Exp)
            nc.scalar.activation(pb[:, qs:], psSB[:, qs:], AF.Exp)
            if kt >= 4:
                # triangular mask on the diagonal 128-col block only
                nc.vector.tensor_mul(pa[:, qs:qs + P], pa[:, qs:qs + P], tri)
                nc.vector.tensor_mul(pb[:, qs:qs + P], pb[:, qs:qs + P], tri)
            nc.tensor.matmul(psA[:, qs:], V_sb[:, kt, 2 * hp, :], pa[:, qs:],
                             start=(kt == 0), stop=(kt == 7))
            nc.tensor.matmul(psB[:, qs:], V_sb[:, kt, 2 * hp + 1, :],
                             pb[:, qs:], start=(kt == 0), stop=(kt == 7))

        # evict unnormalized numerators + denominator rows (short chain)
        nc.vector.tensor_copy(out=a_all[0:64, hp, :], in_=psA[0:64, :])
        btmp = bt_pool.tile([64, TOK], BF16, tag="bt")
        nc.vector.tensor_copy(out=btmp, in_=psB[0:64, :])
        nc.gpsimd.dma_start(out=a_all[64:128, hp, :], in_=btmp)
        h2 = 2 * hp
        dstgA = dst_pool.tile([65, TOK], BF16, tag="dsta")
        dstgB = dst_pool.tile([65, TOK], BF16, tag="dstb")
        nc.vector.tensor_copy(out=dstgA[64:65, 0:TOK], in_=psA[64:65, :])
        nc.vector.tensor_copy(out=dstgB[64:65, 0:TOK], in_=psB[64:65, :])
        nc.gpsimd.dma_start(
            out=den16[h2:h2 + 1, :], in_=dstgA[64:65, 0:TOK])
        nc.gpsimd.dma_start(
            out=den16[h2 + 1:h2 + 2, :], in_=dstgB[64:65, 0:TOK])

    # batched softmax normalization: recip over all 16 heads at once
    with nc.allow_low_precision(reason="softmax denominators tolerate bf16"):
        nc.vector.reciprocal(out=den16, in_=den16)
    for hp in range(8):
        psRB = ps_big.tile([P, TOK], F32, tag="ps")
        nc.tensor.matmul(psRB, sel16[:, hp, :], den16, start=True, stop=True)
        nc.vector.tensor_mul(a_all[:, hp, :], a_all[:, hp, :], psRB)
    if DEBUG:
        nc.gpsimd.dma_start(out=dbg_a, in_=a_all)
        nc.gpsimd.dma_start(out=dbg_dd, in_=den16)

    att_stack.close()  # wq/wk/V/q/k/p/den/psacc dead

    r1_pool = ctx.enter_context(tc.tile_pool(name="r1_pool", bufs=1,
                                             side="right"))
    r1 = r1_pool.tile([P, 8, TOK], BF16)

    def layer_norm(src, dst, g_s, b_s, ln_ps, ln_sb, psSum, psSq):
        """dst = g * (src - mean) / sqrt(std + eps) + b; stats over features
        (partition direction, 8 tiles). psSum/psSq are pre-accumulated by the
        producer loop. Row math runs full-lane on a DMA-scattered [128, 8]."""
        lsc = ln_sb.tile([P, 8], F32, tag="lsc")
        scr = ln_sb.tile([P, 8], F32, tag="scr")
        strow = ln_sb.tile([1, 2, TOK], F32, tag="strow")
        nc.vector.tensor_copy(out=strow[0:1, 0, :], in_=psSum)
        nc.vector.tensor_copy(out=strow[0:1, 1, :], in_=psSq)
        nc.gpsimd.dma_start(out=lsc[:, 0:4], in_=strow[0:1, 0, :])
        nc.gpsimd.dma_start(out=lsc[:, 4:8], in_=strow[0:1, 1, :])
        # mean, E[x^2], var (unbiased), std, 1/sqrt(std+eps)
        nc.vector.tensor_scalar_mul(out=scr[:, 0:4], in0=lsc[:, 0:4],
                                    scalar1=float(1.0 / D))
        nc.vector.tensor_scalar_mul(out=lsc[:, 0:4], in0=lsc[:, 4:8],
                                    scalar1=float(1.0 / D))
        nc.vector.tensor_mul(lsc[:, 4:8], scr[:, 0:4], scr[:, 0:4])
        nc.vector.tensor_sub(lsc[:, 4:8], lsc[:, 0:4], lsc[:, 4:8])
        nc.scalar.activation(lsc[:, 4:8], lsc[:, 4:8], AF.Sqrt,
                             scale=float(D / (D - 1.0)))
        nc.scalar.activation(lsc[:, 4:8], lsc[:, 4:8], AF.Sqrt, bias=eps_c)
        nc.vector.reciprocal(out=scr[:, 4:8], in_=lsc[:, 4:8])
        mrow = ln_sb.tile([1, 2, TOK], F32, tag="mrow")
        nc.gpsimd.dma_start(out=mrow[0:1, 0, :], in_=scr[:, 0:4])
        nc.gpsimd.dma_start(out=mrow[0:1, 1, :], in_=scr[:, 4:8])
        psMR = ln_ps.tile([P, 2, TOK], F32, tag="psmr")
        nc.tensor.matmul(psMR[:, 0, :], _r(ones_row), _r(mrow[0:1, 0, :]),
                         start=True, stop=True)
        nc.tensor.matmul(psMR[:, 1, :], _r(ones_row), _r(mrow[0:1, 1, :]),
                         start=True, stop=True)
        mean_b = ln_sb.tile([P, TOK], BF16, tag="mean_b")
        nc.vector.tensor_copy(out=mean_b, in_=psMR[:, 0, :])
        rs_b = ln_sb.tile([P, TOK], BF16, tag="rs_b")
        nc.vector.tensor_copy(out=rs_b, in_=psMR[:, 1, :])
        for mt in range(8):
            t1 = ln_sb.tile([P, TOK], BF16, tag="t1")
            nc.vector.tensor_sub(t1, src[:, mt, :], mean_b)
            nc.vector.scalar_tensor_tensor(
                out=dst[:, mt, :], in0=t1, scalar=g_s[:, mt:mt + 1],
                in1=rs_b, op0=OP.mult, op1=OP.mult)
            nc.vector.tensor_scalar_add(
                out=dst[:, mt, :], in0=dst[:, mt, :],
                scalar1=b_s[:, mt:mt + 1])

    h1_pool = ctx.enter_context(tc.tile_pool(name="h1_pool", bufs=1))
    h1 = h1_pool.tile([P, 8, TOK], BF16)
    r2y_pool = ctx.enter_context(tc.tile_pool(name="r2y", bufs=1, side="right"))
    r2 = r2y_pool.tile([P, 8, TOK], BF16)
    y = r2y_pool.tile([P, 8, TOK], BF16)

    # ========= attn projection + residual, LN1 stats interleaved =========
    with tc.tile_pool(name="ln1_ps", bufs=1, space="PSUM") as ln1_ps, \
            tc.tile_pool(name="ln1st", bufs=2, space="PSUM") as ln1_st, \
            tc.tile_pool(name="ln1_sb", bufs=2) as ln1_sb:
        psSum = ln1_st.tile([1, TOK], F32, tag="st")
        psSq = ln1_st.tile([1, TOK], F32, tag="st")
        for mt in range(8):
            psO = ps_big.tile([P, TOK], F32, tag="ps")
            for j in range(8):
                nc.tensor.matmul(psO, wat_all[:, mt, j, :], a_all[:, j, :],
                                 start=(j == 0), stop=(j == 7))
            nc.vector.scalar_tensor_tensor(
                out=r1[:, mt, :], in0=psO, scalar=battn_s[:, mt:mt + 1],
                in1=X_f[:, mt, TOK:CTX], op0=OP.add, op1=OP.add)
            sq_t = ln1_sb.tile([P, TOK], BF16, tag="sq")
            nc.vector.tensor_mul(sq_t, r1[:, mt, :], r1[:, mt, :])
            nc.tensor.matmul(psSum, ones_bf, r1[:, mt, :],
                             start=(mt == 0), stop=(mt == 7))
            nc.tensor.matmul(psSq, ones_bf, sq_t,
                             start=(mt == 0), stop=(mt == 7))

        if DEBUG:
            dbg_strow = ln1_sb.tile([1, 2, TOK], F32, tag="dbgrow")
            nc.vector.tensor_copy(out=dbg_strow[0:1, 0, :], in_=psSum)
            nc.vector.tensor_copy(out=dbg_strow[0:1, 1, :], in_=psSq)
            nc.gpsimd.dma_start(out=dbg_st, in_=dbg_strow)
        layer_norm(r1, h1, ln1g_s, ln1b_s, ln1_ps, ln1_sb, psSum, psSq)
        if DEBUG:
            nc.gpsimd.dma_start(out=dbg_r1, in_=r1)
            nc.gpsimd.dma_start(out=dbg_h1, in_=h1)

    # ================= MLP =================
    with tc.tile_pool(name="m1_pool", bufs=1) as m1_pool, \
            tc.tile_pool(name="wfc", bufs=4) as wfc_pool, \
            tc.tile_pool(name="wmlp", bufs=3) as wmlp_pool, \
            tc.tile_pool(name="ln2_ps", bufs=1, space="PSUM") as ln2_ps, \
            tc.tile_pool(name="ln2st", bufs=2, space="PSUM") as ln2_st, \
            tc.tile_pool(name="ln2_sb", bufs=2) as ln2_sb:
        m1 = m1_pool.tile([P, 32, TOK], BF16)
        for mt in range(32):
            wfc_t = wfc_pool.tile([P, 8, P], BF16, tag="wfc")
            nc.sync.dma_start(out=wfc_t, in_=wfc4[mt])
            psF = ps_big.tile([P, TOK], F32, tag="ps")
            for dk in range(8):
                nc.tensor.matmul(psF, wfc_t[:, dk, :], h1[:, dk, :],
                                 start=(dk == 0), stop=(dk == 7))
            nc.scalar.activation(m1[:, mt, :], psF, AF.Relu,
                                 bias=bfc_s[:, mt:mt + 1], scale=1.0)
        psSum2 = ln2_st.tile([1, TOK], F32, tag="st")
        psSq2 = ln2_st.tile([1, TOK], F32, tag="st")
        for mt in range(8):
            wmlp_t = wmlp_pool.tile([P, 32, P], BF16, tag="wmlp")
            nc.sync.dma_start(out=wmlp_t, in_=wmlp4[mt])
            psM = ps_big.tile([P, TOK], F32, tag="ps")
            for k4 in range(32):
                nc.tensor.matmul(psM, wmlp_t[:, k4, :], m1[:, k4, :],
                                 start=(k4 == 0), stop=(k4 == 31))
            nc.vector.scalar_tensor_tensor(
                out=r2[:, mt, :], in0=psM, scalar=bmlp_s[:, mt:mt + 1],
                in1=h1[:, mt, :], op0=OP.add, op1=OP.add)
            sq_t = ln2_sb.tile([P, TOK], BF16, tag="sq")
            nc.vector.tensor_mul(sq_t, r2[:, mt, :], r2[:, mt, :])
            nc.tensor.matmul(psSum2, ones_bf, r2[:, mt, :],
                             start=(mt == 0), stop=(mt == 7))
            nc.tensor.matmul(psSq2, ones_bf, sq_t,
                             start=(mt == 0), stop=(mt == 7))

        y_out = y
        layer_norm(r2, y_out, ln2g_s, ln2b_s, ln2_ps, ln2_sb, psSum2, psSq2)
        for mt in range(8):
            nc.gpsimd.dma_start(out=out[mt], in_=y_out[:, mt, :])

    ctx.close()


_BUILT = None


def _build():
    global _BUILT
    if _BUILT is not None:
        return _BUILT
    nc = bacc.Bacc("TRN2", target_bir_lowering=False, debug=False,
                   enable_asserts=False, num_devices=N_CORES)

    def din(name, shape, dtype=F32):
        return nc.dram_tensor(name, list(shape), dtype, kind="ExternalInput").ap()

    xt = din("xt", (8, P, 8, P), BF16)           # [tt, p, dt, m]
    wq_all = din("wq_all", (P, 8, 8, P), BF16)   # [p, hp, dk, m]
    wk_all = din("wk_all", (P, 8, 8, P), BF16)
    wv3 = din("wv3", (P, 8, D), BF16)            # [p, dk, m]
    b_qkv = din("b_qkv", (3 * D,))
    wat_all = din("wat_all", (P, 8, 8, P), BF16)  # [p, mt, j, m]
    b_attn = din("b_attn_proj", (D,))
    ln1_g = din("ln1_g", (D,))
    ln1_b = din("ln1_b", (D,))
    wfc4 = din("wfc4", (32, P, 8, P), BF16)      # [mt, p, dk, m]
    b_fc = din("b_fc", (4 * D,))
    wmlp4 = din("wmlp4", (8, P, 32, P), BF16)    # [mt, p, k4, m]
    b_mlp = din("b_mlp_proj", (D,))
    ln2_g = din("ln2_g", (D,))
    ln2_b = din("ln2_b", (D,))
    tri = din("tri", (P, P), BF16)               # [m, n] = n >= m
    vcol = din("vcol", (P, 8 * H), BF16)         # denominator column
    bvA = din("bvA", (D,), BF16)                 # v bias, ctx tiles 0..3
    bvB = din("bvB", (D,), BF16)                 # v bias, ctx tiles 4..7
    sel16 = din("sel16", (H, 8, P), BF16)        # head-pair selector
    out_h = nc.dram_tensor("out", [8, P, TOK], BF16, kind="ExternalOutput")
    dbg_a = nc.dram_tensor("dbg_a", [P, 8, TOK], BF16, kind="ExternalOutput")
    dbg_dd = nc.dram_tensor("dbg_dd", [H, TOK], BF16, kind="ExternalOutput")
    dbg_st = nc.dram_tensor("dbg_st", [1, 2, TOK], F32, kind="ExternalOutput")
    dbg_r1 = nc.dram_tensor("dbg_r1", [P, 8, TOK], BF16, kind="ExternalOutput")
    dbg_h1 = nc.dram_tensor("dbg_h1", [P, 8, TOK], BF16, kind="ExternalOutput")

    io = [xt, wq_all, wk_all, wv3, b_qkv, wat_all, b_attn, ln1_g, ln1_b,
          wfc4, b_fc, wmlp4, b_mlp, ln2_g, ln2_b, tri, vcol, bvA, bvB,
          sel16, out_h.ap(), dbg_a.ap(), dbg_dd.ap(),
          dbg_st.ap(), dbg_r1.ap(), dbg_h1.ap()]
    with tile.TileContext(nc) as tc:
        build_block_kernel(nc, tc, io)
    nc.compile()
    _BUILT = nc
    return nc


def _tile4(w, n_in, n_out):
    """[K, M] weight -> [n_out tiles, P, n_in tiles, P]: t4[mt, p, k, m] =
    w[k*P + p, mt*P + m]."""
    K, M = w.shape
    assert K == n_in * P and M == n_out * P
    return np.ascontiguousarray(
        w.reshape(n_in, P, n_out, P).transpose(2, 1, 0, 3))


def _sel16():
    s = np.zeros((H, 8, P), np.float32)
    for hp in range(8):
        s[2 * hp, hp, 0:64] = 1.0
        s[2 * hp + 1, hp, 64:128] = 1.0
    return s


def _wall(w):
    """[K=1024, M=1024] -> [p, mt, k, m] with element = w[k*P+p, mt*P+m]."""
    return np.ascontiguousarray(w.reshape(8, P, 8, P).transpose(1, 2, 0, 3))


def _in_maps(inputs):
    bf = lambda a: np.ascontiguousarray(np.asarray(a, dtype=np.float32)
                                        .astype(ml_dtypes.bfloat16))
    f32 = lambda a: np.ascontiguousarray(np.asarray(a), dtype=np.float32)
    x = np.asarray(inputs["x"], dtype=np.float32)
    w_qkv = np.asarray(inputs["w_qkv"], dtype=np.float32)
    bv = np.asarray(inputs["b_qkv"], dtype=np.float32)[2 * D:]
    shared = {
        "wq_all": bf(_wall(w_qkv[:, 0:D])),
        "wk_all": bf(_wall(w_qkv[:, D:2 * D])),
        "wv3": bf(w_qkv[:, 2 * D:].reshape(8, P, D).transpose(1, 0, 2)),
        "wat_all": bf(_wall(np.asarray(inputs["w_attn_proj"], np.float32))),
        "wfc4": bf(_tile4(np.asarray(inputs["w_fc"], np.float32), 8, 32)),
        "wmlp4": bf(_tile4(np.asarray(inputs["w_mlp_proj"], np.float32), 32, 8)),
        "b_qkv": f32(inputs["b_qkv"]),
        "b_attn_proj": f32(inputs["b_attn_proj"]),
        "ln1_g": f32(inputs["ln1_g"]), "ln1_b": f32(inputs["ln1_b"]),
        "b_fc": f32(inputs["b_fc"]),
        "b_mlp_proj": f32(inputs["b_mlp_proj"]),
        "ln2_g": f32(inputs["ln2_g"]), "ln2_b": f32(inputs["ln2_b"]),
        "tri": bf((np.arange(P)[:, None] <= np.arange(P)[None, :])
                  .astype(np.float32)),
        "sel16": bf(_sel16()),
        "bvB": bf(bv),
    }
    maps = []
    for b in range(B):
        for hh in range(2):
            x_core = np.zeros((CTX, D), np.float32)
            if hh == 1:
                x_core[:TOK] = x[b, :TOK]
            x_core[TOK:] = x[b, hh * TOK:(hh + 1) * TOK]
            xT = x_core.T                      # [D, CTX]
            xt_t = np.ascontiguousarray(       # [tt, p, dt, m]
                xT.reshape(8, P, 8, P).transpose(2, 1, 0, 3))
            vc = np.ones((P, 8, H), np.float32)
            if hh == 0:
                vc[:, 0:4, :] = 0.0
            bvA = bv if hh == 1 else np.zeros_like(bv)
            maps.append({"xt": bf(xt_t), "vcol": bf(vc.reshape(P, 8 * H)),
                         "bvA": bf(bvA), **shared})
    return maps


def run_on_cores(inputs, trace=False, **kwargs):
    """Run the SPMD kernel; returns (full_output, BassKernelResults)."""
    nc = _build()
    maps = _in_maps(inputs)
    res = run_bass_kernel_spmd(nc, maps, core_ids=list(range(N_CORES)),
                               trace=trace, **kwargs)
    out = np.zeros((B, S, D), np.float32)
    for c in range(N_CORES):
        b, hh = divmod(c, 2)
        o = np.asarray(res.results[c]["out"], dtype=np.float32)  # [8, P, TOK]
        out[b, hh * TOK:(hh + 1) * TOK] = o.transpose(2, 0, 1).reshape(TOK, D)
    return out, res


def kernel(**inputs) -> np.ndarray:
    out, _ = run_on_cores(inputs, trace=False)
    return out


# revision 27
# speedup vs baseline: 1.5082x; 1.0553x over previous
"""Trainium2 Bass kernel for a dense transformer block (attention + MLP, 2 LayerNorms).

Sharding: pure data-parallel over 8 cores, one shard per (batch, half-sequence):
core 2*b + h handles queries for tokens [h*512, (h+1)*512) of batch b. Each core
recomputes K/V for its full causal context; no collectives. The junk prefix on
even cores (h=0) is neutralized with per-core data (zeroed V bias + zeroed
denominator column), so no full-tile mask multiplies are needed — only the 8
diagonal [128,128] triangular blocks get a mask multiply.

Everything on the matmul path is bf16 (PE is N-column-bound, so bf16 matches
fp32r peak on big tiles and is 4x faster on the N=128 diagonal tiles, while
halving DMA bytes and doubling DVE throughput). PSUM accumulation is fp32.
LayerNorm row stats are DMA-scattered to [128, x] tiles so all scalar math
runs full-lane; mean/rstd broadcast back via a 1-row PE matmul.
"""

from contextlib import ExitStack

import ml_dtypes
import numpy as np

import concourse.bacc as bacc
import concourse.bass as bass
import concourse.tile as tile
from concourse import mybir
from concourse.bass_utils import run_bass_kernel_spmd

B, S, D, H = 4, 1024, 1024, 16
DH = D // H
EPS = 1e-5
TOK = 512   # queries per core
CTX = 1024  # context tokens per core
P = 128
F32 = mybir.dt.float32
F32R = mybir.dt.float32r
BF16 = mybir.dt.bfloat16
AF = mybir.ActivationFunctionType
OP = mybir.AluOpType

N_CORES = 8
DEBUG = False
QSTART = [0, 0, 0, 0, 0, 128, 256, 384]   # first query col computed per ctx tile
TT_ORDER = [4, 5, 6, 7, 0, 1, 2, 3]       # own-token tiles first (early PE work)


def _r(ap):
    """View an fp32 AP as float32r for full-rate PE matmuls."""
    return ap.bitcast(F32R)


def build_block_kernel(nc, tc, io):
    ctx = ExitStack()
    (xt, wq_all_h, wk_all_h, wv3, b_qkv, wat_all_h, b_attn, ln1_g, ln1_b,
     wfc4, b_fc, wmlp4, b_mlp, ln2_g, ln2_b, maskb_h, vcol_h, bvA_h, bvB_h,
     sel16_h, xo_h, out, dbg_a, dbg_dd, dbg_st, dbg_r1, dbg_h1) = io

    const = ctx.enter_context(tc.tile_pool(name="const", bufs=1))

    ones_row = const.tile([1, P], F32)       # lhsT for row->all-partition bcast
    nc.vector.memset(ones_row, 1.0)
    ones_bf = const.tile([P, 1], BF16)       # lhsT for column-sum stats
    nc.vector.memset(ones_bf, 1.0)
    sel16 = const.tile([H, 8, P], BF16)      # per-head-pair selector lhsT
    eps_c = const.tile([P, 1], F32)
    nc.vector.memset(eps_c, EPS)

    tri = const.tile([P, P], BF16)           # tri[m, n] = n >= m
    vcol = const.tile([P, 8 * H], BF16)      # per-core denominator column
    bvA = const.tile([P, D], BF16)           # v-bias for ctx tiles 0..3
    bvB = const.tile([P, D], BF16)           # v-bias for ctx tiles 4..7

    def load_consts():
        nc.sync.dma_start(out=tri, in_=tri_h)
        nc.sync.dma_start(out=vcol, in_=vcol_h)
        for bv_t, bv_src in ((bvA, bvA_h), (bvB, bvB_h)):
            nc.sync.dma_start(
                out=bv_t,
                in_=bass.AP(tensor=bv_src.tensor, offset=bv_src.offset,
                            ap=[[0, P]] + list(bv_src.ap)))

    def col_param(src_ap, n_tiles, name):
        t = const.tile([P, n_tiles], F32, name=name)
        nc.sync.dma_start(out=t, in_=src_ap.rearrange("(t p) -> p t", p=P))
        return t

    params = {}

    def load_params():
        params["bq_s"] = col_param(b_qkv[0:D], 8, "bq_s")
        bq_sc = const.tile([P, 8], F32)
        nc.vector.tensor_scalar_mul(out=bq_sc, in0=params["bq_s"],
                                    scalar1=float(1.0 / np.sqrt(DH)))
        params["bq_sc"] = bq_sc
        params["bk_s"] = col_param(b_qkv[D:2 * D], 8, "bk_s")
        params["battn_s"] = col_param(b_attn, 8, "battn_s")
        params["ln1g_s"] = col_param(ln1_g, 8, "ln1g_s")
        params["ln1b_s"] = col_param(ln1_b, 8, "ln1b_s")
        params["bfc_s"] = col_param(b_fc, 32, "bfc_s")
        params["bmlp_s"] = col_param(b_mlp, 8, "bmlp_s")
        params["ln2g_s"] = col_param(ln2_g, 8, "ln2g_s")
        params["ln2b_s"] = col_param(ln2_b, 8, "ln2b_s")
        nc.sync.dma_start(out=sel16, in_=sel16_h)

    ps_big = ctx.enter_context(tc.tile_pool(name="ps_big", bufs=4, space="PSUM"))

    xa_pool = ctx.enter_context(tc.tile_pool(name="xa_pool", bufs=1))
    X_f = xa_pool.tile([P, 8, CTX], BF16)        # x^T, feature-major

    a_pool = ctx.enter_context(tc.tile_pool(name="a_pool", bufs=1))
    a_all = a_pool.tile([P, 8, TOK], BF16)       # attention out^T per head-pair

    att_stack = ExitStack()
    wqk_pool = att_stack.enter_context(tc.tile_pool(name="wqk", bufs=1))
    v_pool = att_stack.enter_context(tc.tile_pool(name="v_pool", bufs=1))
    den_pool = att_stack.enter_context(tc.tile_pool(name="den", bufs=1))
    q_pool = att_stack.enter_context(tc.tile_pool(name="q_pool", bufs=2))
    k_pool = att_stack.enter_context(tc.tile_pool(name="k_pool", bufs=2))
    p_pool = att_stack.enter_context(tc.tile_pool(name="p_pool", bufs=8))
    bt_pool = att_stack.enter_context(tc.tile_pool(name="bt_pool", bufs=2))
    dst_pool = att_stack.enter_context(tc.tile_pool(name="dst_pool", bufs=4))
    ps_acc = att_stack.enter_context(
        tc.tile_pool(name="ps_acc", bufs=4, space="PSUM"))

    V_sb = v_pool.tile([P, 8, H, DH + 1], BF16)  # [V | den-col] token-major
    den16 = den_pool.tile([H, TOK], BF16)        # head h denominator on part h

    # ============ phase 0: load x^T + weights, compute V ============
    with tc.tile_pool(name="wv_pool", bufs=1) as wv_pool:
        wv_t = wv_pool.tile([P, 8, D], BF16)
        nc.sync.dma_start(out=wv_t[:, :, 0:TOK], in_=wv3[:, :, 0:TOK])
        for i, tt in enumerate(TT_ORDER):
            eng = nc.sync if i % 2 == 0 else nc.gpsimd
            eng.dma_start(out=X_f[:, :, tt * P:(tt + 1) * P], in_=xt[tt])
        nc.sync.dma_start(out=wv_t[:, :, TOK:], in_=wv3[:, :, TOK:])
        wq_all = wqk_pool.tile([P, 8, 8, P], BF16)
        nc.gpsimd.dma_start(out=wq_all, in_=wq_all_h)
        wk_all = wqk_pool.tile([P, 8, 8, P], BF16)
        nc.gpsimd.dma_start(out=wk_all, in_=wk_all_h)
        load_consts()
        load_params()

        # denominator column (zeroed on prefix tiles for h=0 cores)
        nc.scalar.copy(out=V_sb[:, :, :, DH:DH + 1],
                       in_=vcol.rearrange("p (a b c) -> p a b c", a=8, b=H))

        for tt in TT_ORDER:
            bv_t = bvA if tt < 4 else bvB
            for half in range(2):
                psV = ps_big.tile([P, TOK], F32, tag="ps")
                for dk in range(8):
                    nc.tensor.matmul(psV, X_f[:, dk, tt * P:(tt + 1) * P],
                                     wv_t[:, dk, half * TOK:(half + 1) * TOK],
                                     start=(dk == 0), stop=(dk == 7))
                nc.vector.scalar_tensor_tensor(
                    out=V_sb[:, tt, half * 8:(half + 1) * 8, 0:DH],
                    in0=psV.rearrange("p (h d) -> p h d", d=DH),
                    scalar=0.0, in1=bv_t[:, half * TOK:(half + 1) * TOK]
                    .rearrange("p (h d) -> p h d", d=DH),
                    op0=OP.add, op1=OP.add)

    # wat loads during attention so attn-proj starts immediately after
    wat_pool = ctx.enter_context(tc.tile_pool(name="wat", bufs=1, side="right"))
    wat_all = wat_pool.tile([P, 8, 8, P], BF16)
    nc.scalar.dma_start(out=wat_all, in_=wat_all_h)

    # ============== attention, one head-pair at a time ==============
    for hp in range(8):
        psQ = ps_big.tile([P, TOK], F32, tag="ps")
        for dk in range(8):
            nc.tensor.matmul(psQ, wq_all[:, hp, dk, :], X_f[:, dk, TOK:CTX],
                             start=(dk == 0), stop=(dk == 7))
        q_t = q_pool.tile([P, TOK], BF16, tag="q")
        # fold the 1/sqrt(dh) softmax scale into Q
        nc.vector.tensor_scalar(
            out=q_t, in0=psQ, scalar1=float(1.0 / np.sqrt(DH)),
            scalar2=params["bq_sc"][:, hp:hp + 1], op0=OP.mult, op1=OP.add)

        k_t = k_pool.tile([P, CTX], BF16, tag="k")
        for half in range(2):
            psK = ps_big.tile([P, TOK], F32, tag="ps")
            for dk in range(8):
                nc.tensor.matmul(psK, wk_all[:, hp, dk, :],
                                 X_f[:, dk, half * TOK:(half + 1) * TOK],
                                 start=(dk == 0), stop=(dk == 7))
            nc.vector.tensor_scalar_add(
                out=k_t[:, half * TOK:(half + 1) * TOK], in0=psK,
                scalar1=params["bk_s"][:, hp:hp + 1])

        psA = ps_acc.tile([65, TOK], F32, tag="acc")
        psB = ps_acc.tile([65, TOK], F32, tag="acc")
        for kt in range(8):
            qs = QSTART[kt]
            psSA = ps_big.tile([P, TOK], F32, tag="ps")
            psSB = ps_big.tile([P, TOK], F32, tag="ps")
            nc.tensor.matmul(psSA[:, qs:], k_t[0:64, kt * P:(kt + 1) * P],
                             q_t[0:64, qs:], start=True, stop=True,
                             tile_position=(0, 0))
            nc.tensor.matmul(psSB[:, qs:], k_t[64:128, kt * P:(kt + 1) * P],
                             q_t[64:128, qs:], start=True, stop=True,
                             tile_position=(64, 0))
            pa = p_pool.tile([P, TOK], BF16, tag="p")
            pb = p_pool.tile([P, TOK], BF16, tag="p")
            nc.scalar.activation(pa[:, qs:], psSA[:, qs:], AF.Exp)
            nc.scalar.activation(pb[:, qs:], psSB[:, qs:], AF.Exp)
            if kt >= 4:
                # triangular mask on the diagonal 128-col block only
                nc.vector.tensor_mul(pa[:, qs:qs + P], pa[:, qs:qs + P], tri)
                nc.vector.tensor_mul(pb[:, qs:qs + P], pb[:, qs:qs + P], tri)
            nc.tensor.matmul(psA[:, qs:], V_sb[:, kt, 2 * hp, :], pa[:, qs:],
                             start=(kt == 0), stop=(kt == 7))
            nc.tensor.matmul(psB[:, qs:], V_sb[:, kt, 2 * hp + 1, :],
                             pb[:, qs:], start=(kt == 0), stop=(kt == 7))

        # evict unnormalized numerators + denominator rows (short chain)
        nc.vector.tensor_copy(out=a_all[0:64, hp, :], in_=psA[0:64, :])
        btmp = bt_pool.tile([64, TOK], BF16, tag="bt")
        nc.vector.tensor_copy(out=btmp, in_=psB[0:64, :])
        nc.gpsimd.dma_start(out=a_all[64:128, hp, :], in_=btmp)
        h2 = 2 * hp
        dstgA = dst_pool.tile([65, TOK], BF16, tag="dsta")
        dstgB = dst_pool.tile([65, TOK], BF16, tag="dstb")
        nc.vector.tensor_copy(out=dstgA[64:65, 0:TOK], in_=psA[64:65, :])
        nc.vector.tensor_copy(out=dstgB[64:65, 0:TOK], in_=psB[64:65, :])
        nc.gpsimd.dma_start(
            out=den16[h2:h2 + 1, :], in_=dstgA[64:65, 0:TOK])
        nc.gpsimd.dma_start(
            out=den16[h2 + 1:h2 + 2, :], in_=dstgB[64:65, 0:TOK])

    # batched softmax normalization: recip over all 16 heads at once
    with nc.allow_low_precision(reason="softmax denominators tolerate bf16"):
        nc.vector.reciprocal(out=den16, in_=den16)
    for hp in range(8):
        psRB = ps_big.tile([P, TOK], F32, tag="ps")
        nc.tensor.matmul(psRB, sel16[:, hp, :], den16, start=True, stop=True)
        nc.vector.tensor_mul(a_all[:, hp, :], a_all[:, hp, :], psRB)
    if DEBUG:
        nc.gpsimd.dma_start(out=dbg_a, in_=a_all)
        nc.gpsimd.dma_start(out=dbg_dd, in_=den16)

    att_stack.close()  # wq/wk/V/q/k/p/den/psacc dead

    r1_pool = ctx.enter_context(tc.tile_pool(name="r1_pool", bufs=1,
                                             side="right"))
    r1 = r1_pool.tile([P, 8, TOK], BF16)

    def layer_norm(src, dst, g_s, b_s, ln_ps, ln_sb, psSum, psSq):
        """dst = g * (src - mean) / sqrt(std + eps) + b; stats over features
        (partition direction, 8 tiles). psSum/psSq are pre-accumulated by the
        producer loop. Row math runs full-lane on a DMA-scattered [128, 8]."""
        # row math directly on partition-0 rows: DVE/ACT cost is free-size
        # bound, so [1,512] ops cost the same as [128,4] but skip the DMA
        # scatter/gather roundtrips on the critical path.
        mrow = ln_sb.tile([1, 4, TOK], F32R, tag="mrow")
        nc.vector.tensor_copy(out=mrow[0:1, 2, :], in_=psSum)
        nc.vector.tensor_copy(out=mrow[0:1, 3, :], in_=psSq)
        nc.vector.tensor_scalar_mul(out=mrow[0:1, 0, :], in0=mrow[0:1, 2, :],
                                    scalar1=float(1.0 / D))
        nc.vector.tensor_scalar_mul(out=mrow[0:1, 3, :], in0=mrow[0:1, 3, :],
                                    scalar1=float(1.0 / D))
        nc.vector.tensor_mul(mrow[0:1, 2, :], mrow[0:1, 0, :], mrow[0:1, 0, :])
        nc.vector.tensor_sub(mrow[0:1, 3, :], mrow[0:1, 3, :], mrow[0:1, 2, :])
        # var^-0.25 = exp(-ln(var*c)/4): Ln/Exp/Relu share one ACT table
        nc.scalar.activation(mrow[0:1, 2, :], mrow[0:1, 3, :], AF.Ln,
                             scale=float(D / (D - 1.0)))
        nc.scalar.activation(mrow[0:1, 1, :], mrow[0:1, 2, :], AF.Exp,
                             scale=-0.25)
        psMR = ln_ps.tile([P, 2, TOK], F32, tag="psmr")
        nc.tensor.matmul(psMR[:, 0, :], _r(ones_row), mrow[0:1, 0, :],
                         start=True, stop=True)
        nc.tensor.matmul(psMR[:, 1, :], _r(ones_row), mrow[0:1, 1, :],
                         start=True, stop=True)
        mean_b = ln_sb.tile([P, TOK], BF16, tag="mean_b")
        nc.vector.tensor_copy(out=mean_b, in_=psMR[:, 0, :])
        rs_b = ln_sb.tile([P, TOK], BF16, tag="rs_b")
        nc.vector.tensor_copy(out=rs_b, in_=psMR[:, 1, :])
        for mt in range(8):
            t1 = ln_sb.tile([P, TOK], BF16, tag="t1")
            nc.vector.tensor_sub(t1, src[:, mt, :], mean_b)
            nc.vector.scalar_tensor_tensor(
                out=dst[:, mt, :], in0=t1, scalar=g_s[:, mt:mt + 1],
                in1=rs_b, op0=OP.mult, op1=OP.mult)
            nc.vector.tensor_scalar_add(
                out=dst[:, mt, :], in0=dst[:, mt, :],
                scalar1=b_s[:, mt:mt + 1])

    h1_pool = ctx.enter_context(tc.tile_pool(name="h1_pool", bufs=1))
    h1 = h1_pool.tile([P, 8, TOK], BF16)
    r2y_pool = ctx.enter_context(tc.tile_pool(name="r2y", bufs=1, side="right"))
    r2 = r2y_pool.tile([P, 8, TOK], BF16)
    y = r2y_pool.tile([P, 8, TOK], BF16)

    # ========= attn projection + residual, LN1 stats interleaved =========
    with tc.tile_pool(name="ln1_ps", bufs=1, space="PSUM") as ln1_ps, \
            tc.tile_pool(name="ln1st", bufs=2, space="PSUM") as ln1_st, \
            tc.tile_pool(name="ln1_sb", bufs=2) as ln1_sb:
        psSum = ln1_st.tile([1, TOK], F32, tag="st")
        psSq = ln1_st.tile([1, TOK], F32, tag="st")
        for mt in range(8):
            psO = ps_big.tile([P, TOK], F32, tag="ps")
            for j in range(8):
                nc.tensor.matmul(psO, wat_all[:, mt, j, :], a_all[:, j, :],
                                 start=(j == 0), stop=(j == 7))
            nc.vector.scalar_tensor_tensor(
                out=r1[:, mt, :], in0=psO, scalar=params["battn_s"][:, mt:mt + 1],
                in1=X_f[:, mt, TOK:CTX], op0=OP.add, op1=OP.add)
            sq_t = ln1_sb.tile([P, TOK], BF16, tag="sq")
            nc.vector.tensor_mul(sq_t, r1[:, mt, :], r1[:, mt, :])
            nc.tensor.matmul(psSum, ones_bf, r1[:, mt, :],
                             start=(mt == 0), stop=(mt == 7))
            nc.tensor.matmul(psSq, ones_bf, sq_t,
                             start=(mt == 0), stop=(mt == 7))

        if DEBUG:
            dbg_strow = ln1_sb.tile([1, 2, TOK], F32, tag="dbgrow")
            nc.vector.tensor_copy(out=dbg_strow[0:1, 0, :], in_=psSum)
            nc.vector.tensor_copy(out=dbg_strow[0:1, 1, :], in_=psSq)
            nc.gpsimd.dma_start(out=dbg_st, in_=dbg_strow)
        layer_norm(r1, h1, params["ln1g_s"], params["ln1b_s"], ln1_ps, ln1_sb, psSum, psSq)
        if DEBUG:
            nc.gpsimd.dma_start(out=dbg_r1, in_=r1)
            nc.gpsimd.dma_start(out=dbg_h1, in_=h1)

    # ================= MLP =================
    with tc.tile_pool(name="m1_pool", bufs=1) as m1_pool, \
            tc.tile_pool(name="wfc", bufs=4) as wfc_pool, \
            tc.tile_pool(name="wmlp", bufs=3) as wmlp_pool, \
            tc.tile_pool(name="ln2_ps", bufs=1, space="PSUM") as ln2_ps, \
            tc.tile_pool(name="ln2st", bufs=2, space="PSUM") as ln2_st, \
            tc.tile_pool(name="ln2_sb", bufs=2) as ln2_sb:
        m1 = m1_pool.tile([P, 32, TOK], BF16)
        for mt in range(32):
            wfc_t = wfc_pool.tile([P, 8, P], BF16, tag="wfc")
            nc.gpsimd.dma_start(out=wfc_t, in_=wfc4[mt])
            psF = ps_big.tile([P, TOK], F32, tag="ps")
            for dk in range(8):
                nc.tensor.matmul(psF, wfc_t[:, dk, :], h1[:, dk, :],
                                 start=(dk == 0), stop=(dk == 7))
            nc.scalar.activation(m1[:, mt, :], psF, AF.Relu,
                                 bias=params["bfc_s"][:, mt:mt + 1], scale=1.0)
        psSum2 = ln2_st.tile([1, TOK], F32, tag="st")
        psSq2 = ln2_st.tile([1, TOK], F32, tag="st")
        for mt in range(8):
            wmlp_t = wmlp_pool.tile([P, 32, P], BF16, tag="wmlp")
            nc.gpsimd.dma_start(out=wmlp_t, in_=wmlp4[mt])
            psM = ps_big.tile([P, TOK], F32, tag="ps")
            for k4 in range(32):
                nc.tensor.matmul(psM, wmlp_t[:, k4, :], m1[:, k4, :],
                                 start=(k4 == 0), stop=(k4 == 31))
            nc.vector.scalar_tensor_tensor(
                out=r2[:, mt, :], in0=psM, scalar=params["bmlp_s"][:, mt:mt + 1],
                in1=h1[:, mt, :], op0=OP.add, op1=OP.add)
            sq_t = ln2_sb.tile([P, TOK], BF16, tag="sq")
            nc.vector.tensor_mul(sq_t, r2[:, mt, :], r2[:, mt, :])
            nc.tensor.matmul(psSum2, ones_bf, r2[:, mt, :],
                             start=(mt == 0), stop=(mt == 7))
            nc.tensor.matmul(psSq2, ones_bf, sq_t,
                             start=(mt == 0), stop=(mt == 7))

        y_out = y
        layer_norm(r2, y_out, params["ln2g_s"], params["ln2b_s"], ln2_ps, ln2_sb, psSum2, psSq2)
        for mt in range(8):
            nc.gpsimd.dma_start(out=out[mt], in_=y_out[:, mt, :])

    ctx.close()


_BUILT = None


def _build():
    global _BUILT
    if _BUILT is not None:
        return _BUILT
    nc = bacc.Bacc("TRN2", target_bir_lowering=False, debug=False,
                   enable_asserts=False, num_devices=N_CORES)

    def din(name, shape, dtype=F32):
        return nc.dram_tensor(name, list(shape), dtype, kind="ExternalInput").ap()

    xt = din("xt", (8, P, 8, P), BF16)           # [tt, p, dt, m]
    wq_all = din("wq_all", (P, 8, 8, P), BF16)   # [p, hp, dk, m]
    wk_all = din("wk_all", (P, 8, 8, P), BF16)
    wv3 = din("wv3", (P, 8, D), BF16)            # [p, dk, m]
    b_qkv = din("b_qkv", (3 * D,))
    wat_all = din("wat_all", (P, 8, 8, P), BF16)  # [p, mt, j, m]
    b_attn = din("b_attn_proj", (D,))
    ln1_g = din("ln1_g", (D,))
    ln1_b = din("ln1_b", (D,))
    wfc4 = din("wfc4", (32, P, 8, P), BF16)      # [mt, p, dk, m]
    b_fc = din("b_fc", (4 * D,))
    wmlp4 = din("wmlp4", (8, P, 32, P), BF16)    # [mt, p, k4, m]
    b_mlp = din("b_mlp_proj", (D,))
    ln2_g = din("ln2_g", (D,))
    ln2_b = din("ln2_b", (D,))
    tri = din("tri", (P, P), BF16)               # [m, n] = n >= m
    vcol = din("vcol", (P, 8 * H), BF16)         # denominator column
    bvA = din("bvA", (D,), BF16)                 # v bias, ctx tiles 0..3
    bvB = din("bvB", (D,), BF16)                 # v bias, ctx tiles 4..7
    sel16 = din("sel16", (H, 8, P), BF16)        # head-pair selector
    out_h = nc.dram_tensor("out", [8, P, TOK], BF16, kind="ExternalOutput")
    dbg_a = nc.dram_tensor("dbg_a", [P, 8, TOK], BF16, kind="ExternalOutput")
    dbg_dd = nc.dram_tensor("dbg_dd", [H, TOK], BF16, kind="ExternalOutput")
    dbg_st = nc.dram_tensor("dbg_st", [1, 2, TOK], F32, kind="ExternalOutput")
    dbg_r1 = nc.dram_tensor("dbg_r1", [P, 8, TOK], BF16, kind="ExternalOutput")
    dbg_h1 = nc.dram_tensor("dbg_h1", [P, 8, TOK], BF16, kind="ExternalOutput")

    io = [xt, wq_all, wk_all, wv3, b_qkv, wat_all, b_attn, ln1_g, ln1_b,
          wfc4, b_fc, wmlp4, b_mlp, ln2_g, ln2_b, tri, vcol, bvA, bvB,
          sel16, out_h.ap(), dbg_a.ap(), dbg_dd.ap(),
          dbg_st.ap(), dbg_r1.ap(), dbg_h1.ap()]
    with tile.TileContext(nc) as tc:
        build_block_kernel(nc, tc, io)
    nc.compile()
    _BUILT = nc
    return nc


def _tile4(w, n_in, n_out):
    """[K, M] weight -> [n_out tiles, P, n_in tiles, P]: t4[mt, p, k, m] =
    w[k*P + p, mt*P + m]."""
    K, M = w.shape
    assert K == n_in * P and M == n_out * P
    return np.ascontiguousarray(
        w.reshape(n_in, P, n_out, P).transpose(2, 1, 0, 3))


def _sel16():
    s = np.zeros((H, 8, P), np.float32)
    for hp in range(8):
        s[2 * hp, hp, 0:64] = 1.0
        s[2 * hp + 1, hp, 64:128] = 1.0
    return s


def _wall(w):
    """[K=1024, M=1024] -> [p, mt, k, m] with element = w[k*P+p, mt*P+m]."""
    return np.ascontiguousarray(w.reshape(8, P, 8, P).transpose(1, 2, 0, 3))


def _in_maps(inputs):
    bf = lambda a: np.ascontiguousarray(np.asarray(a, dtype=np.float32)
                                        .astype(ml_dtypes.bfloat16))
    f32 = lambda a: np.ascontiguousarray(np.asarray(a), dtype=np.float32)
    x = np.asarray(inputs["x"], dtype=np.float32)
    w_qkv = np.asarray(inputs["w_qkv"], dtype=np.float32)
    bv = np.asarray(inputs["b_qkv"], dtype=np.float32)[2 * D:]
    shared = {
        "wq_all": bf(_wall(w_qkv[:, 0:D])),
        "wk_all": bf(_wall(w_qkv[:, D:2 * D])),
        "wv3": bf(w_qkv[:, 2 * D:].reshape(8, P, D).transpose(1, 0, 2)),
        "wat_all": bf(_wall(np.asarray(inputs["w_attn_proj"], np.float32))),
        "wfc4": bf(_tile4(np.asarray(inputs["w_fc"], np.float32), 8, 32)),
        "wmlp4": bf(_tile4(np.asarray(inputs["w_mlp_proj"], np.float32), 32, 8)),
        "b_qkv": f32(inputs["b_qkv"]),
        "b_attn_proj": f32(inputs["b_attn_proj"]),
        "ln1_g": f32(inputs["ln1_g"]), "ln1_b": f32(inputs["ln1_b"]),
        "b_fc": f32(inputs["b_fc"]),
        "b_mlp_proj": f32(inputs["b_mlp_proj"]),
        "ln2_g": f32(inputs["ln2_g"]), "ln2_b": f32(inputs["ln2_b"]),
        "tri": bf((np.arange(P)[:, None] <= np.arange(P)[None, :])
                  .astype(np.float32)),
        "sel16": bf(_sel16()),
        "bvB": bf(bv),
    }
    maps = []
    for b in range(B):
        for hh in range(2):
            x_core = np.zeros((CTX, D), np.float32)
            if hh == 1:
                x_core[:TOK] = x[b, :TOK]
            x_core[TOK:] = x[b, hh * TOK:(hh + 1) * TOK]
            xT = x_core.T                      # [D, CTX]
            xt_t = np.ascontiguousarray(       # [tt, p, dt, m]
                xT.reshape(8, P, 8, P).transpose(2, 1, 0, 3))
            vc = np.ones((P, 8, H), np.float32)
            if hh == 0:
                vc[:, 0:4, :] = 0.0
            bvA = bv if hh == 1 else np.zeros_like(bv)
            maps.append({"xt": bf(xt_t), "vcol": bf(vc.reshape(P, 8 * H)),
                         "bvA": bf(bvA), **shared})
    return maps


def run_on_cores(inputs, trace=False, **kwargs):
    """Run the SPMD kernel; returns (full_output, BassKernelResults)."""
    nc = _build()
    maps = _in_maps(inputs)
    res = run_bass_kernel_spmd(nc, maps, core_ids=list(range(N_CORES)),
                               trace=trace, **kwargs)
    out = np.zeros((B, S, D), np.float32)
    for c in range(N_CORES):
        b, hh = divmod(c, 2)
        o = np.asarray(res.results[c]["out"], dtype=np.float32)  # [8, P, TOK]
        out[b, hh * TOK:(hh + 1) * TOK] = o.transpose(2, 0, 1).reshape(TOK, D)
    return out, res


def kernel(**inputs) -> np.ndarray:
    out, _ = run_on_cores(inputs, trace=False)
    return out
